# revision 39
# baseline (speedup 1.0000x reference)
"""Trainium2 Bass kernel for nn_CrossAttn_65214783422649.

Algebraic reduction: softmax over R followed by mean over R is identically 1/R,
so the attention branch (Wq, Wk, energy, softmax) cancels:

    sims[i, c] = (a_c + b_i) . cs_c / (||a_c + b_i|| * ||cs_c||)
      a_c  = (gamma/R) * sum_t mask * leaky(cap_c @ Wvt.T + bvt)
      b_i  = mean_r leaky(img_i @ Wvi.T + bvi)
      cs_c = masked-sum_t cap_c          (the /lens cancels inside l2norm)

Sharding: captions 8-way (binpacked by lens), images 8-way for b, with a bf16
AllGather of b shards (+ 0.5|b|^2).

v2 schedule (from the v1 trace):
  * dummy PE matmuls on a zeroed tile warm the HAM clock gate during the DMA
    lead-in, so the img phase runs at 2.4 GHz instead of 1.2 GHz.
  * img loads are chunked along the contraction dim on both HWDGE queues and
    the img matmuls run k-major across all it-tiles (5 concurrent PSUM
    groups), so the PE tracks the DMA chunk-by-chunk instead of stalling for
    the whole img working set.
  * cap-phase PE/ACT work is explicitly ordered AFTER the img phase + bnat
    chain (sync=False same-engine deps), so the AllGather input is ready as
    soon as the img phase ends instead of drifting ~25us later.
  * the collective trigger rides the (otherwise idle) vector queue.
  * post-AG tail: b / 0.5|b|^2 gathered, PE transposes rebuild bT, and the
    final num/den matmuls are slimmed down.
"""

import numpy as np
import ml_dtypes

import concourse.bass as bass
import concourse.mybir as mybir
import concourse.tile as tile
from concourse import bacc
from concourse.bass import ds, ts
from concourse.bass_utils import run_bass_kernel_spmd
from concourse.tile import add_dep_helper

F32 = mybir.dt.float32
BF16 = mybir.dt.bfloat16
F8 = mybir.dt.float8e4
DR = mybir.MatmulPerfMode.DoubleRow
AF = mybir.ActivationFunctionType
WS = 64.0                      # fp8 weight prescale (leaky is homogeneous)

N_CORES = 8
B_I, B_C, R, T, D = 128, 128, 36, 64, 1024
C_SH = B_C // N_CORES          # 16 captions per core
I_SH = B_I // N_CORES          # 16 images per core
IMG_TOK = I_SH * R             # 576 image tokens per core
IMG_PAD = 640                  # padded to 5 * 128
KT = D // 128                  # 8 contraction tiles
IT = IMG_PAD // 128            # 5 image token tiles
NEG_SLOPE = 0.1
AGP = D                        # AllGather payload: just the b rows (bf16)
N_DUMMY = 10                   # PE warm-up matmuls during the DMA lead-in
WAVE = 5                       # concurrent PSUM groups per k-major wave

_CACHE: dict = {}


def _build(CT: int, with_bias: bool):
    """CT = number of 128-token caption tiles after host packing."""
    CAP_TOK = CT * 128
    nc = bacc.Bacc("TRN2", target_bir_lowering=False, debug=False,
                   num_devices=N_CORES)

    imgT_d = nc.dram_tensor("imgT", [D, IMG_PAD], F8, kind="ExternalInput")
    wviT_d = nc.dram_tensor("wviT", [D, D], F8, kind="ExternalInput")
    capT_d = nc.dram_tensor("capT", [D, CAP_TOK], F8, kind="ExternalInput")
    wvtT_d = nc.dram_tensor("wvtT", [D, D], F8, kind="ExternalInput")
    cap_d = nc.dram_tensor("cap", [CAP_TOK, D], BF16, kind="ExternalInput")
    om_a_d = nc.dram_tensor("om_a", [CAP_TOK, C_SH], BF16, kind="ExternalInput")
    om_b_d = nc.dram_tensor("om_b", [IMG_PAD, I_SH], BF16, kind="ExternalInput")
    idb_d = nc.dram_tensor("idb", [128, 128], BF16, kind="ExternalInput")
    gam_d = nc.dram_tensor("gam16", [C_SH, 1], F32, kind="ExternalInput")
    if with_bias:
        bft_d = nc.dram_tensor("bias_vt", [128, D], F32, kind="ExternalInput")
        bfi_d = nc.dram_tensor("bias_vi", [128, D], F32, kind="ExternalInput")
    sims_d = nc.dram_tensor("sims", [C_SH, B_I], F32, kind="ExternalOutput")

    with tile.TileContext(nc) as tc:
        with (
            tc.tile_pool(name="const", bufs=1) as const,
            tc.tile_pool(name="wt", bufs=1) as wtp,
            tc.tile_pool(name="xt", bufs=1) as xtp,
            tc.tile_pool(name="vtx", bufs=4) as vtxp,
            tc.tile_pool(name="gpool", bufs=1) as gp,
            tc.tile_pool(name="small", bufs=1) as sp,
            tc.tile_pool(name="ps_mm", bufs=WAVE, space="PSUM") as ps_mm,
            tc.tile_pool(name="ps_acc", bufs=2, space="PSUM") as ps_acc,
            tc.tile_pool(name="ps_tr", bufs=1, space="PSUM") as ps_tr,
            tc.tile_pool(name="dram", bufs=1, space="DRAM") as dram,
        ):
            # ---- PE warm-up: dummy matmuls on a zeroed tile (no DMA deps).
            # Keeps the HAM activity window busy from the moment the PE
            # sequencer is live, so real img matmuls run at 2.4 GHz.
            zsrc = const.tile([128, 512], BF16, tag="zsrc")
            nc.vector.memset(zsrc[:], 0.0)
            for w in range(N_DUMMY):
                pmw = ps_mm.tile([128, 512], F32, tag="mm", name=f"pwarm{w}")
                nc.tensor.matmul(pmw[:], zsrc[:, 0:128], zsrc[:],
                                 start=True, stop=True)

            # ---- img-phase loads: single-k chunks alternating between both
            # HWDGE queues, so both queues carry ONLY img bytes first (full
            # HBM bandwidth) and chunks land in k-order for the k-major waves.
            imgT_s = xtp.tile([128, KT, IMG_PAD], F8, tag="imgT")
            wvi_s = wtp.tile([128, KT, D], F8, tag="wt_vi")
            for k in range(KT):
                qa, qb = (nc.sync, nc.scalar) if k % 2 == 0 else \
                         (nc.scalar, nc.sync)
                qa.dma_start(out=imgT_s[:, k, :], in_=imgT_d[ts(k, 128), :])
                qb.dma_start(out=wvi_s[:, k, :], in_=wviT_d[ts(k, 128), :])
            om_b_s = const.tile([128, IT, I_SH], BF16, tag="om_b")
            nc.gpsimd.dma_start(
                out=om_b_s[:], in_=om_b_d.rearrange("(a p) c -> p a c", p=128))
            identb = const.tile([128, 128], BF16, tag="idb")
            nc.gpsimd.dma_start(out=identb[:], in_=idb_d[:, :])
            gam16 = const.tile([C_SH, 1], F32, tag="gam16")
            nc.gpsimd.dma_start(out=gam16[:], in_=gam_d[:, :])
            if with_bias:
                bias_vi = const.tile([128, D], F32, tag="bias_vi")
                nc.gpsimd.dma_start(out=bias_vi[:], in_=bfi_d[:, :])
                bias_vt = const.tile([128, D], F32, tag="bias_vt")
                nc.gpsimd.dma_start(out=bias_vt[:], in_=bft_d[:, :])
            ones_row = const.tile([1, 128], BF16, tag="ones_row")
            nc.vector.memset(ones_row[:], 1.0)
            ones_col = const.tile([128, 1], BF16, tag="ones_col")
            nc.vector.memset(ones_col[:], 1.0)

            # ---- cap-phase loads: behind the img loads on the same queues
            # (FIFO per queue => img bytes get full HBM bandwidth first),
            # balanced and ordered by when the cap waves consume them.
            capT_s = xtp.tile([128, KT, CAP_TOK], F8, tag="capT")
            wvt_s = wtp.tile([128, KT, D], F8, tag="wt_vt")
            cnat_all = xtp.tile([128, CT, D], BF16, tag="capnat")
            nc.sync.dma_start(
                out=capT_s[:, ds(0, 4), :].opt(),
                in_=capT_d[ds(0, 512), :].rearrange("(a p) c -> p a c", p=128))
            nc.scalar.dma_start(
                out=wvt_s[:, ds(0, 4), :].opt(),
                in_=wvtT_d[ds(0, 512), :].rearrange("(kt p) e -> p kt e", p=128))
            nc.sync.dma_start(
                out=wvt_s[:, ds(4, 4), :].opt(),
                in_=wvtT_d[ds(512, 512), :].rearrange(
                    "(kt p) e -> p kt e", p=128))
            nc.scalar.dma_start(
                out=capT_s[:, ds(4, 4), :].opt(),
                in_=capT_d[ds(512, 512), :].rearrange(
                    "(a p) c -> p a c", p=128))
            CTA = (CT + 1) // 2
            nc.sync.dma_start(
                out=cnat_all[:, 0:CTA, :].opt(),
                in_=cap_d[0:CTA * 128, :].rearrange("(a p) d -> p a d", p=128))
            if CT > CTA:
                nc.scalar.dma_start(
                    out=cnat_all[:, CTA:CT, :].opt(),
                    in_=cap_d[CTA * 128:CAP_TOK, :].rearrange(
                        "(a p) d -> p a d", p=128))
            om_a_s = const.tile([128, CT, C_SH], BF16, tag="om_a")
            nc.gpsimd.dma_start(
                out=om_a_s[:], in_=om_a_d.rearrange("(a p) c -> p a c", p=128))

            # ---- img matmul phase: k-major waves (one per dh half) across
            # all IT token tiles, so the PE chases the DMA chunk-by-chunk.
            # Matmul outputs must start at partition 0/32/64, so packed
            # accumulator banks place 16-row groups at those offsets:
            #   ps_b bank:  [0:16]=b dh0, [32:48]=b dh1, [64:80]=capsum dh1
            #   ps_ac bank: [0:16]=a dh0, [32:48]=a dh1, [64:80]=capsum dh0
            ps_b = ps_acc.tile([128, 512], F32, tag="acc", name="ps_b")
            last_om_b = None
            for dh in range(2):
                # The dh1 wave borrows the (idle until the tail) transpose
                # bank for its first group, so its k-matmuls start without
                # waiting for a dh0 evacuation to free a ring slot.
                pms = []
                for it in range(IT):
                    if dh == 1 and it == 0:
                        pms.append(ps_tr.tile([128, 512], F32, tag="tr",
                                              name="pmi0_1"))
                    else:
                        pms.append(ps_mm.tile([128, 512], F32, tag="mm",
                                              name=f"pmi{it}_{dh}"))
                for k in range(0, KT, 2):
                    for it in range(IT):
                        nc.tensor.matmul(pms[it][:],
                                         imgT_s[:, ds(k, 2), ts(it, 128)],
                                         wvi_s[:, ds(k, 2), ds(dh * 512, 512)],
                                         start=(k == 0), stop=(k == KT - 2),
                                         perf_mode=DR)
                for it in range(IT):
                    if with_bias:
                        nc.vector.tensor_add(pms[it][:], pms[it][:],
                                             bias_vi[:, ds(dh * 512, 512)])
                    vimg = vtxp.tile([128, 512], BF16, tag="vtx",
                                     name=f"vimg{it}_{dh}")
                    if dh == 1 and it % 2 == 1 and not with_bias:
                        # Split the last wave's evacuations across ACT and
                        # DVE: leaky(x) = max(x/WS, 0.1x/WS) on the vector
                        # engine, halving the serial chain in front of om_b.
                        tmp = vtxp.tile([128, 512], BF16, tag="vtx",
                                        name=f"vtmp{it}")
                        nc.vector.tensor_scalar_mul(tmp[:], pms[it][:],
                                                    NEG_SLOPE / WS)
                        nc.vector.scalar_tensor_tensor(
                            out=vimg[:], in0=pms[it][:], scalar=1.0 / WS,
                            in1=tmp[:], op0=mybir.AluOpType.mult,
                            op1=mybir.AluOpType.max)
                    else:
                        nc.scalar.activation(vimg[:], pms[it][:],
                                             AF.Prelu, alpha=NEG_SLOPE,
                                             scale=1.0 / WS)
                    last_om_b = nc.tensor.matmul(
                        ps_b[ds(dh * 32, 16), :], om_b_s[:, it, :], vimg[:],
                        start=(it == 0), stop=(it == IT - 1))

            # ---- bnat: b shard (scaled 1/R) straight off PSUM. |b|^2 is
            # recomputed from the gathered bT post-AllGather, keeping this
            # pre-AllGather chain minimal.
            bnat = sp.tile([I_SH, AGP], BF16, tag="bnat")
            bnat_acts = []
            for dh in range(2):
                a1 = nc.scalar.activation(bnat[:, ds(dh * 512, 512)],
                                          ps_b[ds(dh * 32, 16), :],
                                          AF.Identity, scale=1.0 / R)
                bnat_acts.append(a1)
            ag_in = dram.tile([I_SH, AGP], BF16, tag="ag_in")
            ag_out = dram.tile([B_I, AGP], BF16, addr_space="Shared", tag="ag_out")
            ag_dma = nc.sync.dma_start(out=ag_in[:], in_=bnat[:])
            nc.gpsimd.collective_compute(
                "AllGather",
                mybir.AluOpType.bypass,
                replica_groups=[list(range(N_CORES))],
                ins=[ag_in[:].opt()],
                outs=[ag_out[:].opt()],
            )

            # ---- cap matmul phase: k-major waves of <=WAVE (ct, dh) groups.
            # Each dh half's evac / transpose / zpack tail is emitted as soon
            # as that half completes, so it pipelines under the next wave.
            ps_ac = ps_acc.tile([128, 512], F32, tag="acc", name="ps_ac")
            a_s = sp.tile([C_SH, D], BF16, tag="a_s")
            cs_s = sp.tile([C_SH, D], BF16, tag="cs_s")
            acT = gp.tile([128, KT, 32], BF16, tag="acT")
            zpack = gp.tile([128, KT, 80], BF16, tag="zpack")
            ps_sc = ps_acc.tile([80, 1], F32, tag="acc", name="ps_sc")

            def cap_dh_evac(dh):
                """ACT-side evac of a / capsum for one dh half."""
                cm_bank = ps_ac if dh == 0 else ps_b
                nc.scalar.activation(a_s[:, ds(dh * 512, 512)],
                                     ps_ac[ds(dh * 32, 16), :],
                                     AF.Identity, scale=gam16[:])
                nc.scalar.activation(cs_s[:, ds(dh * 512, 512)],
                                     cm_bank[ds(64, 16), :],
                                     AF.Copy)

            def cap_dh_tail(dh):
                """PE/DVE-side tail for one dh half: transposes into acT,
                zpack partial products, and the sc partial reduction."""
                g = dh
                for src, co, nm in ((a_s, 0, "a"), (cs_s, 16, "c")):
                    pst = ps_tr.tile([128, 4 * C_SH], BF16, tag="tr",
                                     name=f"pq{nm}{g}")
                    for j in range(4):
                        k = 4 * g + j
                        nc.tensor.transpose(pst[:, ts(j, C_SH)],
                                            src[:, ts(k, 128)],
                                            identb[0:C_SH, 0:C_SH])
                    nc.vector.tensor_copy(
                        acT[:, ds(4 * g, 4), ds(co, 16)].opt(), pst[:])
                kv = ds(4 * g, 4)
                nc.vector.tensor_mul(zpack[:, kv, 0:C_SH].opt(),
                                     acT[:, kv, 0:16], acT[:, kv, 16:32])
                nc.vector.tensor_mul(zpack[:, kv, 32:32 + C_SH].opt(),
                                     acT[:, kv, 0:16], acT[:, kv, 0:16])
                nc.vector.tensor_mul(zpack[:, kv, 64:64 + C_SH].opt(),
                                     acT[:, kv, 16:32], acT[:, kv, 16:32])
                for j in range(4):
                    k = 4 * g + j
                    nc.tensor.matmul(ps_sc[:], zpack[:, k, :], ones_col[:],
                                     start=(k == 0), stop=(k == KT - 1))

            groups = [(ct, dh) for dh in range(2) for ct in range(CT)]
            waves = [groups[i:i + WAVE] for i in range(0, len(groups), WAVE)]
            last_wave = waves[-1]
            cap_gate_mms = []   # first mm of each cap PSUM group
            cap_acts = []
            a_mms, cm_mms = [], []
            pending_tail = []
            for wave in waves:
                pms = {g: ps_mm.tile([128, 512], F32, tag="mm",
                                     name=f"pmc{g[0]}_{g[1]}") for g in wave}
                for k in range(0, KT, 2):
                    for g in wave:
                        ct, dh = g
                        mm = nc.tensor.matmul(
                            pms[g][:], capT_s[:, ds(k, 2), ts(ct, 128)],
                            wvt_s[:, ds(k, 2), ds(dh * 512, 512)],
                            start=(k == 0), stop=(k == KT - 2),
                            perf_mode=DR)
                        if k == 0:
                            cap_gate_mms.append(mm)
                for dh_done in pending_tail:
                    cap_dh_tail(dh_done)
                pending_tail = []
                for g in wave:
                    ct, dh = g
                    if with_bias:
                        nc.vector.tensor_add(pms[g][:], pms[g][:],
                                             bias_vt[:, ds(dh * 512, 512)])
                    vtxt = vtxp.tile([128, 512], BF16, tag="vtx",
                                     name=f"vtxt{ct}_{dh}")
                    if wave is last_wave and ct % 2 == 1 and not with_bias:
                        tmp = vtxp.tile([128, 512], BF16, tag="vtx",
                                        name=f"vttmp{ct}_{dh}")
                        nc.vector.tensor_scalar_mul(tmp[:], pms[g][:],
                                                    NEG_SLOPE / WS)
                        act = nc.vector.scalar_tensor_tensor(
                            out=vtxt[:], in0=pms[g][:], scalar=1.0 / WS,
                            in1=tmp[:], op0=mybir.AluOpType.mult,
                            op1=mybir.AluOpType.max)
                    else:
                        act = nc.scalar.activation(vtxt[:], pms[g][:],
                                                   AF.Prelu, alpha=NEG_SLOPE,
                                                   scale=1.0 / WS)
                        cap_acts.append(act)
                    a_mms.append(nc.tensor.matmul(
                        ps_ac[ds(dh * 32, 16), :], om_a_s[:, ct, :], vtxt[:],
                        start=(ct == 0), stop=(ct == CT - 1)))
                    cm_bank = ps_ac if dh == 0 else ps_b
                    cm_mms.append(nc.tensor.matmul(
                        cm_bank[ds(64, 16), :], om_a_s[:, ct, :],
                        cnat_all[:, ct, ds(dh * 512, 512)],
                        start=(ct == 0), stop=(ct == CT - 1)))
                    if ct == CT - 1:
                        cap_dh_evac(dh)
                        pending_tail.append(dh)
            for dh_done in pending_tail:
                cap_dh_tail(dh_done)

            # Order the cap phase strictly after the img phase on each engine
            # so the AllGather input never drifts late. sync=False edges are
            # advisory only (the v2 trace shows the scheduler ignoring them),
            # so these use real semaphore deps.
            for mm in cap_gate_mms:
                add_dep_helper(mm.ins, last_om_b.ins, sync=True,
                               reason="cap mm after img phase")
            for mm in (a_mms[0], a_mms[CT], cm_mms[0], cm_mms[CT]):
                add_dep_helper(mm.ins, last_om_b.ins, sync=True,
                               reason="cap acc mm after img phase")
            for act in cap_acts:
                add_dep_helper(act.ins, bnat_acts[-1].ins, sync=True,
                               reason="cap act after bnat evac")

            sc_s = sp.tile([80, 1], F32, tag="sc_s")
            nc.vector.tensor_copy(sc_s[:], ps_sc[:])
            sqq = sp.tile([C_SH, 1], F32, tag="sqq")
            nc.scalar.activation(sqq[:], sc_s[64:64 + C_SH, :], AF.Sqrt)
            shat = sp.tile([C_SH, 1], F32, tag="shat")
            nc.vector.reciprocal(shat[:], sqq[:])

            # ---- post-AllGather: bT, then |b|^2 row from bT squares.
            # bfull lands in two halves so the first transpose group overlaps
            # the second half's DMA.
            bfull = gp.tile([B_I, D], BF16, tag="bfull")
            nc.sync.dma_start(out=bfull[:, 0:512], in_=ag_out[:, 0:512])
            nc.sync.dma_start(out=bfull[:, 512:D], in_=ag_out[:, 512:D])

            bT = gp.tile([128, KT, B_I], BF16, tag="bT")
            for g in range(2):
                pst = ps_tr.tile([128, 512], BF16, tag="tr", name=f"pb{g}")
                for j in range(4):
                    k = 4 * g + j
                    nc.tensor.transpose(pst[:, ts(j, 128)],
                                        bfull[:, ts(k, 128)], identb[:])
                nc.vector.tensor_copy(bT[:, ds(4 * g, 4), :].opt(), pst[:])

            # |b|^2 per image straight from bfull rows, in parallel (DVE)
            # with the PE transposes.
            bsqf = gp.tile([B_I, D], BF16, tag="bsqf")
            nc.vector.tensor_mul(bsqf[:], bfull[:], bfull[:])
            nb_col = sp.tile([B_I, 1], F32, tag="nb_col")
            nc.vector.reduce_sum(nb_col[:], bsqf[:], axis=mybir.AxisListType.X)
            nb_colh = sp.tile([B_I, 1], BF16, tag="nb_colh")
            nc.vector.tensor_copy(nb_colh[:], nb_col[:])
            ps_nb = ps_tr.tile([1, 512], BF16, tag="tr", name="ps_nb")
            nc.tensor.transpose(ps_nb[:, 0:B_I], nb_colh[:], identb[:])
            nb_row = sp.tile([1, B_I], BF16, tag="nb_row")
            nc.vector.tensor_copy(nb_row[:], ps_nb[:, 0:B_I])

            # ---- similarity assembly: g1 rows [0:16] = a.b + 0.5|b|^2 (the
            # trailing halves_row matmul supplies 0.5|b|^2); g2 rows [32:48]
            # = cs.b. The a.b / cs.b k-matmuls are emitted first so they
            # start as soon as each bT block lands.
            halves_row = const.tile([1, C_SH], BF16, tag="halves_row")
            nc.vector.memset(halves_row[:], 0.5)
            ps_g = ps_acc.tile([48, B_I], F32, tag="acc", name="ps_g")
            for k in range(KT):
                nc.tensor.matmul(ps_g[0:C_SH, :], acT[:, k, 0:16], bT[:, k, :],
                                 start=(k == 0), stop=False)
                nc.tensor.matmul(ps_g[ds(32, 16), :], acT[:, k, 16:32],
                                 bT[:, k, :],
                                 start=(k == 0), stop=(k == KT - 1))
            nc.tensor.matmul(ps_g[0:C_SH, :], halves_row[:], nb_row[:],
                             start=False, stop=True)
            den = sp.tile([C_SH, B_I], F32, tag="den")
            nc.scalar.activation(den[:], ps_g[0:C_SH, :], AF.Sqrt, scale=2.0,
                                 bias=sc_s[32:32 + C_SH, :])
            rden = sp.tile([C_SH, B_I], F32, tag="rden")
            nc.vector.reciprocal(rden[:], den[:])

            num = sp.tile([C_SH, B_I], F32, tag="num")
            nc.vector.tensor_scalar(
                out=num[:], in0=ps_g[ds(32, 16), :], scalar1=sc_s[0:C_SH, :],
                scalar2=shat[:], op0=mybir.AluOpType.add,
                op1=mybir.AluOpType.mult)
            sims_s = sp.tile([C_SH, B_I], F32, tag="sims_s")
            nc.vector.tensor_mul(sims_s[:], num[:], rden[:])
            nc.sync.dma_start(out=sims_d[:, :], in_=sims_s[:])

    nc.compile()
    return nc


def _get_nc(CT: int, with_bias: bool):
    key = (CT, with_bias)
    if key not in _CACHE:
        _CACHE[key] = _build(CT, with_bias)
    return _CACHE[key]


def _balance_captions(lens):
    """Assign 16 captions to each of 8 cores, minimizing the max token sum
    (greedy LPT with per-core cardinality cap). Returns [8][C_SH] index array."""
    order = np.argsort(-lens, kind="stable")
    sums = np.zeros(N_CORES, np.int64)
    counts = np.zeros(N_CORES, np.int64)
    assign = [[] for _ in range(N_CORES)]
    for idx in order:
        open_cores = [m for m in range(N_CORES) if counts[m] < C_SH]
        m = min(open_cores, key=lambda m: (sums[m], m))
        assign[m].append(int(idx))
        sums[m] += int(lens[idx])
        counts[m] += 1
    return np.array(assign, np.int64)


def _host_prep(inputs):
    bf = ml_dtypes.bfloat16
    cap_embed = np.asarray(inputs["cap_embed"], dtype=np.float32)
    img_embed = np.asarray(inputs["img_embed"], dtype=np.float32)
    lens = np.asarray(inputs["lens"]).astype(np.int64)
    wvt = np.asarray(inputs["Wvt"], dtype=np.float32)
    wvi = np.asarray(inputs["Wvi"], dtype=np.float32)
    bvt = np.asarray(inputs["bvt"], dtype=np.float32).reshape(1, D)
    bvi = np.asarray(inputs["bvi"], dtype=np.float32).reshape(1, D)
    with_bias = bool(bvt.any() or bvi.any())
    gamma = float(np.asarray(inputs["gamma_img"]).reshape(-1)[0])

    assign = _balance_captions(lens)
    max_tok = int(lens[assign].sum(axis=1).max())
    CT = max(1, -(-max_tok // 128))
    CAP_TOK = CT * 128

    f8 = ml_dtypes.float8_e4m3
    wvtT = np.ascontiguousarray((wvt.T * WS).astype(f8))
    wviT = np.ascontiguousarray((wvi.T * WS).astype(f8))
    om_b = np.zeros((IMG_PAD, I_SH), bf)
    om_b[:IMG_TOK] = np.repeat(np.eye(I_SH, dtype=bf), R, axis=0)
    identb = np.eye(128, dtype=bf)
    gam16 = np.full((C_SH, 1), gamma / R, np.float32)
    if with_bias:
        bias_vt = np.ascontiguousarray(np.repeat(bvt, 128, axis=0))
        bias_vi = np.ascontiguousarray(np.repeat(bvi, 128, axis=0))

    in_maps = []
    for m in range(N_CORES):
        idxs = assign[m]
        cap = np.zeros((CAP_TOK, D), np.float32)
        om_a = np.zeros((CAP_TOK, C_SH), bf)
        pos = 0
        for c, idx in enumerate(idxs):
            n = int(lens[idx])
            cap[pos:pos + n] = cap_embed[idx, :n]
            om_a[pos:pos + n, c] = 1.0
            pos += n
        img = np.zeros((IMG_PAD, D), np.float32)
        img[:IMG_TOK] = img_embed[m * I_SH:(m + 1) * I_SH].reshape(IMG_TOK, D)
        im = {
            "cap": np.ascontiguousarray(cap.astype(bf)),
            "capT": np.ascontiguousarray(cap.T.astype(f8)),
            "imgT": np.ascontiguousarray(img.T.astype(f8)),
            "wvtT": wvtT,
            "wviT": wviT,
            "om_a": om_a,
            "om_b": om_b,
            "idb": identb,
            "gam16": gam16,
        }
        if with_bias:
            im["bias_vt"] = bias_vt
            im["bias_vi"] = bias_vi
        in_maps.append(im)
    return in_maps, CT, with_bias, assign


def _unshard(res, assign):
    sims = np.empty((B_I, B_C), np.float32)
    for m in range(N_CORES):
        sims[:, assign[m]] = res.results[m]["sims"].T
    return sims


def kernel(**inputs) -> np.ndarray:
    in_maps, CT, with_bias, assign = _host_prep(inputs)
    nc = _get_nc(CT, with_bias)
    res = run_bass_kernel_spmd(nc, in_maps, core_ids=list(range(N_CORES)))
    return _unshard(res, assign)


def run_traced(trace_cores=None, **inputs):
    """For test.py: same as kernel() but with NTFF tracing enabled."""
    in_maps, CT, with_bias, assign = _host_prep(inputs)
    nc = _get_nc(CT, with_bias)
    res = run_bass_kernel_spmd(nc, in_maps, core_ids=list(range(N_CORES)),
                               trace=True, trace_cores=trace_cores)
    return _unshard(res, assign), res
